# revision 8
# baseline (speedup 1.0000x reference)
"""Trainium2 Bass kernel for AttnApply (sliding-window weighted sum).

out[b, t, c] = sum_i padded[b, t+i, c] * weights[b, t, i]   (T=11, D=5 zero pad)

Strategy
--------
Pure data parallel over batch: 8 cores x 4 batches each.

Per core, the windowed sum is a banded matrix multiply on the TensorEngine:
for each time block of M=118 output rows,

    out[t0+m, c] = sum_k band[k, m] * in_pad[t0+k, c],   k in [0, 128)

with band[k, m] = w[t0+m, k-m] for 0 <= k-m < T (zero elsewhere).  K = M+T-1
= 128 exactly fills the PE contraction dim -> ONE fp32 matmul per block.
Band matrices are built host-side (cheap scatter of the small weights tensor)
and streamed from DRAM; inputs are host zero-padded so edge blocks need no
special casing.

Blocks are processed J=7 at a time ("supertile") so each DMA moves ~0.5-1 MB:
 - input supertile  [128, J*C]  (2 DMAs: 118-row body + 10-row halo), on SP
 - band supertile   [128, J*M]  (1 contiguous DMA), on SP
 - 7 matmuls into one PSUM tile [118, J*C] (each output slice is bank-aligned)
 - 1 VectorE copy PSUM -> SBUF
 - output store DMA on ScalarE's HWDGE queue
Separate issue engines per stream keep one stream's semaphore waits from
head-of-line-blocking another stream's queue.
"""

import numpy as np

import concourse.bass as bass  # noqa: F401  (engine handles hang off nc)
import concourse.mybir as mybir
import concourse.tile as tile
from concourse import bacc
from concourse.bass_utils import run_bass_kernel_spmd

B, L, C, T = 32, 4096, 256, 11
D = T // 2
N_CORES = 8
B_LOC = B // N_CORES            # 4 batches per core
M = 118                         # output rows per matmul block
K = M + T - 1                   # 128 = contraction rows per block
NBLK = -(-L // M)               # 35 blocks per batch
J = 7                           # blocks per supertile
NSUP = NBLK // J                # 5 supertiles per batch
SUP = M * J                     # 826 output rows per supertile
# padded input rows; NSUP*SUP + M keeps the halo-load rearrange view
# (rows t0+M .. t0+M+SUP) in bounds for the last supertile
LPAD = NSUP * SUP + M           # 4248

_CACHE: dict = {}
LAST_RESULT = None  # BassKernelResults of the most recent run (for test.py)


def _build_nc(repeat: int = 1, bench: bool = False):
    """Build the bass program. `repeat` re-runs the whole body N times and
    `bench=True` uses internal zero-filled DRAM inputs/outputs with only a
    tiny external "tick" output — both used only for benchmarking; the
    grading path uses repeat=1, bench=False."""
    nc = bacc.Bacc(
        "TRN2",
        target_bir_lowering=False,
        debug=False,
        num_devices=N_CORES,
    )
    if bench:
        inp = nc.dram_tensor("in_int", [B_LOC, LPAD, C], mybir.dt.float32).ap()
        band = nc.dram_tensor(
            "band_int", [B_LOC, NSUP, K, J * M], mybir.dt.float32
        ).ap()
        out = nc.dram_tensor("out_int", [B_LOC, L, C], mybir.dt.float32).ap()
        tick = nc.dram_tensor(
            "tick", [1, C], mybir.dt.float32, kind="ExternalOutput"
        ).ap()
    else:
        inp = nc.dram_tensor(
            "in_pad", [B_LOC, LPAD, C], mybir.dt.float32, kind="ExternalInput"
        ).ap()
        band = nc.dram_tensor(
            "band", [B_LOC, NSUP, K, J * M], mybir.dt.float32, kind="ExternalInput"
        ).ap()
        out = nc.dram_tensor(
            "out", [B_LOC, L, C], mybir.dt.float32, kind="ExternalOutput"
        ).ap()
        tick = None

    with tile.TileContext(nc) as tc:
        with (
            tc.tile_pool(name="inp", bufs=3) as in_pool,
            tc.tile_pool(name="bnd", bufs=3) as bd_pool,
            tc.tile_pool(name="outp", bufs=3) as o_pool,
            tc.tile_pool(name="ps", bufs=2, space="PSUM") as ps_pool,
        ):
            if bench:
                # zero-fill internal inputs once per run (outside the repeat
                # loop; cancelled by the delta-timing method anyway)
                with tc.tile_pool(name="z", bufs=1) as z_pool:
                    z = z_pool.tile([128, 1024], mybir.dt.float32, tag="z")
                    nc.gpsimd.memset(z[:, :], 0.0)
                    for b in range(B_LOC):
                        for r0 in range(0, LPAD, 128):
                            cnt = min(128, LPAD - r0)
                            nc.sync.dma_start(
                                out=inp[b, r0 : r0 + cnt, :], in_=z[:cnt, :C]
                            )
                        for s in range(NSUP):
                            nc.sync.dma_start(
                                out=band[b, s], in_=z[:, : J * M]
                            )

            for _rep in range(repeat):
                for b in range(B_LOC):
                    for s in range(NSUP):
                        t0 = s * SUP
                        # ---- loads (SP HWDGE queue) ----
                        in_t = in_pool.tile([K, J * C], mybir.dt.float32, tag="in")
                        nc.sync.dma_start(
                            out=in_t[:M, :].rearrange("p (j c) -> p j c", j=J),
                            in_=inp[b, t0 : t0 + SUP, :].rearrange(
                                "(j k) c -> k j c", j=J
                            ),
                        )
                        nc.sync.dma_start(
                            out=in_t[M:K, :].rearrange("p (j c) -> p j c", j=J),
                            in_=inp[b, t0 + M : t0 + M + SUP, :].rearrange(
                                "(j k) c -> k j c", j=J
                            )[: K - M],
                        )
                        bd_t = bd_pool.tile([K, J * M], mybir.dt.float32, tag="bd")
                        nc.sync.dma_start(out=bd_t[:, :], in_=band[b, s])

                        # ---- 7 matmuls into one PSUM tile ----
                        ps = ps_pool.tile([M, J * C], mybir.dt.float32, tag="ps")
                        for jj in range(J):
                            nc.tensor.matmul(
                                ps[:, jj * C : (jj + 1) * C],
                                bd_t[:, jj * M : (jj + 1) * M],
                                in_t[:, jj * C : (jj + 1) * C],
                                start=True,
                                stop=True,
                            )

                        # ---- PSUM -> SBUF (VectorE) ----
                        o_t = o_pool.tile([M, J * C], mybir.dt.float32, tag="o")
                        nc.vector.tensor_copy(out=o_t[:, :], in_=ps[:, :])

                        # ---- store (ACT HWDGE queue) ----
                        rows = min(SUP, L - t0)
                        jfull = rows // M            # full blocks in this sup
                        if jfull:
                            nc.scalar.dma_start(
                                out=out[b, t0 : t0 + jfull * M, :].rearrange(
                                    "(j m) c -> m j c", j=jfull
                                ),
                                in_=o_t[:, : jfull * C].rearrange(
                                    "p (j c) -> p j c", j=jfull
                                ),
                            )
                        mrem = rows - jfull * M      # partial tail block rows
                        if mrem:
                            nc.scalar.dma_start(
                                out=out[b, t0 + jfull * M : t0 + rows, :],
                                in_=o_t[:mrem, jfull * C : (jfull + 1) * C],
                            )
                if tick is not None:
                    nc.sync.dma_start(out=tick[:, :], in_=o_t[:1, :C])
    nc.compile()
    return nc


def _prep_core(x: np.ndarray, w: np.ndarray):
    """x: [B_LOC, L, C] f32, w: [B_LOC, L, T] f32 -> (in_pad, band)."""
    in_pad = np.zeros((B_LOC, LPAD, C), np.float32)
    in_pad[:, D : D + L, :] = x
    band = np.zeros((B_LOC, NBLK, K, M), np.float32)
    jj, mm = np.meshgrid(np.arange(NBLK), np.arange(M), indexing="ij")
    tt = jj * M + mm
    v = tt < L
    jv, mv_, tv = jj[v], mm[v], tt[v]
    for tau in range(T):
        band[:, jv, mv_ + tau, mv_] = w[:, tv, tau]
    # regroup into supertile layout [B_LOC, NSUP, K, J*M]
    band = np.ascontiguousarray(
        band.reshape(B_LOC, NSUP, J, K, M).transpose(0, 1, 3, 2, 4)
    ).reshape(B_LOC, NSUP, K, J * M)
    return in_pad, band


def kernel(inputs: np.ndarray, weights: np.ndarray) -> np.ndarray:
    global LAST_RESULT
    inputs = np.ascontiguousarray(np.asarray(inputs, dtype=np.float32))
    weights = np.ascontiguousarray(np.asarray(weights, dtype=np.float32))
    assert inputs.shape == (B, L, C) and weights.shape == (B, L, T)

    if "nc" not in _CACHE:
        _CACHE["nc"] = _build_nc()
    nc = _CACHE["nc"]

    in_maps = []
    for c in range(N_CORES):
        sl = slice(c * B_LOC, (c + 1) * B_LOC)
        ip, bd = _prep_core(inputs[sl], weights[sl])
        in_maps.append({"in_pad": ip, "band": bd})

    res = run_bass_kernel_spmd(nc, in_maps, core_ids=list(range(N_CORES)))
    LAST_RESULT = res
    return np.concatenate([r["out"] for r in res.results], axis=0)


# revision 9
# speedup vs baseline: 5.8482x; 5.8482x over previous
"""Trainium2 Bass kernel for AttnApply (sliding-window weighted sum).

out[b, t, c] = sum_i padded[b, t+i, c] * weights[b, t, i]   (T=11, D=5 zero pad)

Strategy
--------
Pure data parallel over batch: 8 cores x 4 batches each.

Per core, the windowed sum is a banded matrix multiply on the TensorEngine.
For a time block of M=118 output rows starting at t0 (K = M+T-1 = 128):

    out[t0+m, c] = sum_k band[k, m] * in_pad[t0+k, c],   k in [0, 128)

with band[k, m] = w[t0+m, k-m] for 0 <= k-m < T (zero elsewhere); in_pad is
host zero-padded so edge blocks need no special casing.  Band matrices are
built host-side (cheap scatter of the small weights tensor).

The matmul runs with the INPUT tile as the stationary operand and the band as
the moving operand, producing the TRANSPOSED output in PSUM:

    psum[c, m] = sum_k in_pad[t0+k, c] * band[k, m]

so PSUM partitions are channels (two 128-channel halves) and the free dim is
time.  Channel-major output means each partition's store is a long contiguous
run in a [C, L] DRAM tensor (host un-transposes at the end) — measured ~5x
faster than time-major stores, which degrade to 1KB-per-descriptor writes
(~60 GB/s vs ~310+ GB/s on this part).

Layout per supertile of J=7 blocks:
 - 7 per-block input loads [128, C] (contiguous, SP queue)
 - 1 band load [128, J*M] (contiguous, ACT queue)
 - 14 matmuls (7 blocks x 2 channel halves) into psum [128, J*128]
   (block stride padded 118->128 so every matmul output is bank-aligned)
 - 2 VectorE copies compact psum -> SBUF [128, J*118]
 - 2 column-major stores [128, 826] (one per half; split SP/ACT queues)
"""

import numpy as np

import concourse.bass as bass  # noqa: F401  (engine handles hang off nc)
import concourse.mybir as mybir
import concourse.tile as tile
from concourse import bacc
from concourse.bass_utils import run_bass_kernel_spmd

B, L, C, T = 32, 4096, 256, 11
D = T // 2
N_CORES = 8
B_LOC = B // N_CORES            # 4 batches per core
M = 118                         # output rows per matmul block
K = M + T - 1                   # 128 = contraction rows per block
NBLK = -(-L // M)               # 35 blocks per batch
J = 7                           # blocks per supertile
NSUP = NBLK // J                # 5 supertiles per batch
SUP = M * J                     # 826 output rows per supertile
MP = 128                        # padded per-block psum stride (bank aligned)
LPAD = (NBLK - 1) * M + K       # 4140 padded input rows

_CACHE: dict = {}
LAST_RESULT = None  # BassKernelResults of the most recent run (for test.py)


def _build_nc(repeat: int = 1, bench: bool = False):
    """Build the bass program. `repeat` re-runs the whole body N times and
    `bench=True` uses internal zero-filled DRAM inputs/outputs with only a
    tiny external "tick" output — both used only for benchmarking; the
    grading path uses repeat=1, bench=False."""
    nc = bacc.Bacc(
        "TRN2",
        target_bir_lowering=False,
        debug=False,
        num_devices=N_CORES,
    )
    if bench:
        inp = nc.dram_tensor("in_int", [B_LOC, LPAD, C], mybir.dt.float32).ap()
        band = nc.dram_tensor(
            "band_int", [B_LOC, NSUP, K, J * M], mybir.dt.float32
        ).ap()
        outT = nc.dram_tensor("outT_int", [B_LOC, C, L], mybir.dt.float32).ap()
        tick = nc.dram_tensor(
            "tick", [1, C], mybir.dt.float32, kind="ExternalOutput"
        ).ap()
    else:
        inp = nc.dram_tensor(
            "in_pad", [B_LOC, LPAD, C], mybir.dt.float32, kind="ExternalInput"
        ).ap()
        band = nc.dram_tensor(
            "band", [B_LOC, NSUP, K, J * M], mybir.dt.float32, kind="ExternalInput"
        ).ap()
        outT = nc.dram_tensor(
            "outT", [B_LOC, C, L], mybir.dt.float32, kind="ExternalOutput"
        ).ap()
        tick = None

    with tile.TileContext(nc) as tc:
        with (
            tc.tile_pool(name="inp", bufs=6) as in_pool,
            tc.tile_pool(name="bnd", bufs=3) as bd_pool,
            tc.tile_pool(name="outp", bufs=3) as o_pool,
            tc.tile_pool(name="ps", bufs=4, space="PSUM") as ps_pool,
        ):
            if bench:
                # back every DRAM page with zeros once per run so reads are
                # real HBM traffic (unbacked-page reads measure absurdly
                # fast and would not represent the grading path)
                with tc.tile_pool(name="z", bufs=1) as z_pool:
                    z = z_pool.tile([K, SUP], mybir.dt.float32, tag="z")
                    nc.gpsimd.memset(z[:, :], 0.0)
                    for b in range(B_LOC):
                        for r0 in range(0, LPAD, K):
                            cnt = min(K, LPAD - r0)
                            nc.sync.dma_start(
                                out=inp[b, r0 : r0 + cnt, :], in_=z[:cnt, :C]
                            )
                        for s in range(NSUP):
                            nc.sync.dma_start(out=band[b, s], in_=z[:, : J * M])
                        for ch in range(2):
                            for s in range(NSUP):
                                lo, hi = s * SUP, min((s + 1) * SUP, L)
                                nc.sync.dma_start(
                                    out=outT[b, ch * 128 : (ch + 1) * 128, lo:hi],
                                    in_=z[:, : hi - lo],
                                )

            for _rep in range(repeat):
                for b in range(B_LOC):
                    for s in range(NSUP):
                        t0 = s * SUP
                        # ---- band load (ACT HWDGE queue) ----
                        bd_t = bd_pool.tile([K, J * M], mybir.dt.float32, tag="bd")
                        nc.scalar.dma_start(out=bd_t[:, :], in_=band[b, s])

                        # ---- per-block input loads (SP HWDGE queue) ----
                        in_ts = []
                        for jj in range(J):
                            tb = t0 + jj * M
                            in_t = in_pool.tile([K, C], mybir.dt.float32, tag="in")
                            nc.sync.dma_start(
                                out=in_t[:, :], in_=inp[b, tb : tb + K, :]
                            )
                            in_ts.append(in_t)

                        # ---- matmuls: psum[c, m] per channel half ----
                        pss = []
                        for ch in range(2):
                            ps = ps_pool.tile(
                                [128, J * MP], mybir.dt.float32, tag="ps"
                            )
                            for jj in range(J):
                                nc.tensor.matmul(
                                    ps[:, jj * MP : jj * MP + M],
                                    in_ts[jj][:, ch * 128 : (ch + 1) * 128],
                                    bd_t[:, jj * M : (jj + 1) * M],
                                    start=True,
                                    stop=True,
                                )
                            pss.append(ps)

                        # ---- compact copy + column-major store ----
                        rows = min(SUP, L - t0)
                        for ch in range(2):
                            o_t = o_pool.tile([128, SUP], mybir.dt.float32, tag="o")
                            nc.vector.tensor_copy(
                                out=o_t[:, :].rearrange("p (j m) -> p j m", j=J),
                                in_=pss[ch]
                                .rearrange("p (j m) -> p j m", j=J)[:, :, :M],
                            )
                            eng = nc.sync if ch == 0 else nc.scalar
                            eng.dma_start(
                                out=outT[b, ch * 128 : (ch + 1) * 128, t0 : t0 + rows],
                                in_=o_t[:, :rows],
                            )
                if tick is not None:
                    # flush both HWDGE queues: same-queue reads complete only
                    # after all prior writes on that queue
                    fl = o_pool.tile([2, C], mybir.dt.float32, tag="fl")
                    nc.sync.dma_start(out=fl[0:1, :], in_=outT[0, 0:1, 0:C])
                    nc.scalar.dma_start(out=fl[1:2, :], in_=outT[0, 128:129, 0:C])
                    nc.sync.dma_start(out=tick[:, :], in_=fl[0:1, :])
                    nc.sync.dma_start(out=tick[:, 0:C], in_=fl[1:2, :])
    nc.compile()
    return nc


def _prep_core(x: np.ndarray, w: np.ndarray):
    """x: [B_LOC, L, C] f32, w: [B_LOC, L, T] f32 -> (in_pad, band)."""
    in_pad = np.zeros((B_LOC, LPAD, C), np.float32)
    in_pad[:, D : D + L, :] = x
    band = np.zeros((B_LOC, NBLK, K, M), np.float32)
    jj, mm = np.meshgrid(np.arange(NBLK), np.arange(M), indexing="ij")
    tt = jj * M + mm
    v = tt < L
    jv, mv_, tv = jj[v], mm[v], tt[v]
    for tau in range(T):
        band[:, jv, mv_ + tau, mv_] = w[:, tv, tau]
    # regroup into supertile layout [B_LOC, NSUP, K, J*M]
    band = np.ascontiguousarray(
        band.reshape(B_LOC, NSUP, J, K, M).transpose(0, 1, 3, 2, 4)
    ).reshape(B_LOC, NSUP, K, J * M)
    return in_pad, band


def kernel(inputs: np.ndarray, weights: np.ndarray) -> np.ndarray:
    global LAST_RESULT
    inputs = np.ascontiguousarray(np.asarray(inputs, dtype=np.float32))
    weights = np.ascontiguousarray(np.asarray(weights, dtype=np.float32))
    assert inputs.shape == (B, L, C) and weights.shape == (B, L, T)

    if "nc" not in _CACHE:
        _CACHE["nc"] = _build_nc()
    nc = _CACHE["nc"]

    in_maps = []
    for c in range(N_CORES):
        sl = slice(c * B_LOC, (c + 1) * B_LOC)
        ip, bd = _prep_core(inputs[sl], weights[sl])
        in_maps.append({"in_pad": ip, "band": bd})

    res = run_bass_kernel_spmd(nc, in_maps, core_ids=list(range(N_CORES)))
    LAST_RESULT = res
    # outputs come back channel-major [B_LOC, C, L]; un-transpose on host
    return np.ascontiguousarray(
        np.concatenate(
            [r["outT"].transpose(0, 2, 1) for r in res.results], axis=0
        )
    )
